# revision 20
# baseline (speedup 1.0000x reference)
"""Trainium2 Bass kernel for nn_CompressorModel (block decompression + linear head).

The reference computes, per sample b:
    y[b] = W . vec(stack_ch(lhs[r] @ block[r,c] @ rhs[c])) + bias
which is linear in x.  We fold (lhs, rhs, W) into a single effective weight
vector W_eff[768] on the host (fp64), reducing the device work to a pure
memory-bound matvec:  y = x.reshape(B, 768) @ W_eff + bias.

Device strategy (pure data parallel, batch sharded 8 ways). Per core the
shard [4096, 768] is viewed as [512, 6144] (partition line = 8 consecutive
rows, 24KB contiguous DRAM) and split into 32 "units" (one unit = 768 elems
on 128 partitions = 512 rows' worth of one row-position = 384KB).

Pipeline (raw bass; this walrus build rejects >1 sync-wait per instruction,
so every cross-engine dep is a standalone wait_ge on its own semaphore):
  SP     : HWDGE DMAs: w [128,768], bias lanes, then x tiles on a tapered
           schedule (small tiles at the edges to shrink startup/tail).
  VectorE: one tensor_mul per tile: prod[*, k, 0:768] = x * w(broadcast),
           writing into a 769-lane-strided product ring; lane 768 holds the
           bias (preloaded once), so the row dot product + bias is exactly
           the 769-lane sum.
  ScalarE: one activation(Copy, accum_out) per unit over 769 lanes ->
           res[:, unit] = sum = y row value (bias included). No adds, no
           self-waits on the critical path.
  GPSIMD : output DMAs [128, 8] per partition-block, off the compute path.
"""

import numpy as np

B = 32768
N_CORES = 8
B_PER = B // N_CORES          # 4096 rows per core
F = 768                       # 3*16*16 features per row
L = F + 1                     # product lanes per unit (768 + bias lane)
RPP = 8                       # rows packed per partition line
NPL = B_PER // RPP            # 512 partition lines per core
LINE = RPP * F                # 6144 elems per partition line
P = 128                       # SBUF partitions
UNITS = 32                    # units per core (one unit = [128, 768])
RING = 16                     # ring capacity in units (x and product rings)
# Tapered tile schedule in units; sums to 32, no tile crosses an 8-unit
# partition block, ring placement (g % 16) never wraps.
TILES = [1, 2, 4, 1, 4, 4, 4, 4, 4, 2, 1, 1]
NXSEM = 4                     # rotating x-DMA completion sems
NOSEM = 2                     # rotating out-DMA completion sems

_cache = {}


def _fold_weights(lhs, rhs, W):
    """W_eff[ch, r*8+p, c*8+q] = sum_{P,Q} lhs[r,P,p]*rhs[c,q,Q]*W[0, ch*1024+(r*16+P)*32+(c*16+Q)]"""
    Wb = np.asarray(W, np.float64).reshape(3, 2, 16, 2, 16)  # [ch, r, P, c, Q]
    weff = np.einsum(
        "rPp,cqQ,nrPcQ->nrpcq",
        np.asarray(lhs, np.float64),
        np.asarray(rhs, np.float64),
        Wb,
    )
    return np.ascontiguousarray(weff.reshape(F)).astype(np.float32)


def _build_program(reps=1):
    """Build the per-core program. reps>1 streams the same shard `reps` times
    (identical output, used only for wall-clock timing harnesses)."""
    key = ("nc", reps)
    if key in _cache:
        return _cache[key]
    from concourse import bass, mybir

    f32 = mybir.dt.float32
    nc = bass.Bass("TRN2", target_bir_lowering=False, debug=False)
    xs = nc.dram_tensor("xs", [NPL, LINE], f32, kind="ExternalInput").ap()
    wf = nc.dram_tensor("wf", [P, F], f32, kind="ExternalInput").ap()
    bs = nc.dram_tensor("bs", [P, RING], f32, kind="ExternalInput").ap()
    ys = nc.dram_tensor("ys", [NPL, RPP], f32, kind="ExternalOutput").ap()

    w_t = nc.alloc_sbuf_tensor("w_t", [P, F], f32).ap()
    xb = nc.alloc_sbuf_tensor("xb", [P, RING * F], f32).ap()
    pb = nc.alloc_sbuf_tensor("pb", [P, RING * L], f32).ap()
    res = nc.alloc_sbuf_tensor("res", [P, UNITS], f32).ap()

    pbv = pb.rearrange("p (s l) -> p s l", s=RING)

    # --- static schedule tables (global over reps) ---
    tiles = []  # (t, g_global, k)
    g = 0
    for r in range(reps):
        for k in TILES:
            tiles.append((len(tiles), g, k))
            g += k
    n_blocks = 4 * reps
    T_UNITS = UNITS * reps

    def xs_src(gg, k):
        u = gg % UNITS
        bl, c = divmod(u, RPP)
        return xs[bl * P : (bl + 1) * P, c * F : (c + k) * F]

    def xb_slot(gg, k):
        rs = gg % RING
        return xb[:, rs * F : (rs + k) * F]

    with (
        nc.Block() as block,
        nc.semaphore("s_w") as s_w,
        nc.semaphore("s_b") as s_b,
        nc.semaphore("s_x0") as s_x0,
        nc.semaphore("s_x1") as s_x1,
        nc.semaphore("s_x2") as s_x2,
        nc.semaphore("s_x3") as s_x3,
        nc.semaphore("s_v") as s_v,
        nc.semaphore("s_a") as s_a,
        nc.semaphore("s_of") as s_of,
        nc.semaphore("s_o0") as s_o0,
        nc.semaphore("s_o1") as s_o1,
    ):
        s_x = [s_x0, s_x1, s_x2, s_x3]
        s_o = [s_o0, s_o1]

        @block.sync
        def _(sync: bass.BassEngine):
            sync.dma_start(out=w_t, in_=wf).then_inc(s_w, 16)
            first = True
            for t, gg, k in tiles:
                if t >= NXSEM:
                    # updater order: previous DMA on this sem lane completed
                    sync.wait_ge(s_x[t % NXSEM], 16 * (t // NXSEM))
                if gg >= RING:
                    # DVE consumed the units previously occupying this range
                    sync.wait_ge(s_v, gg + k - RING)
                sync.dma_start(out=xb_slot(gg, k), in_=xs_src(gg, k)).then_inc(
                    s_x[t % NXSEM], 16
                )
                if first:
                    # bias lanes land right after the first x tile
                    first = False
                    with nc.allow_non_contiguous_dma(
                        reason="tiny one-time bias lanes"
                    ):
                        sync.dma_start(
                            out=pbv[:, :, F : F + 1], in_=bs
                        ).then_inc(s_b, 16)
            bl = n_blocks - 1
            sync.wait_ge(s_a, RPP * (bl + 1))
            if bl >= NOSEM:
                sync.wait_ge(s_o[bl % NOSEM], 16 * (bl // NOSEM))
            h = (bl % 4) * RPP
            sync.dma_start(
                out=ys[(bl % 4) * P : (bl % 4 + 1) * P, :],
                in_=res[:, h : h + RPP],
            ).then_inc(s_of, 16)
            n_pool = n_blocks - 1
            sync.wait_ge(s_of, 16)
            sync.wait_ge(s_o0, 16 * ((n_pool + 1) // 2))
            sync.wait_ge(s_o1, 16 * (n_pool // 2))

        @block.vector
        def _(vec: bass.BassEngine):
            vec.wait_ge(s_w, 16)
            vec.wait_ge(s_b, 16)
            for t, gg, k in tiles:
                vec.wait_ge(s_x[t % NXSEM], 16 * (t // NXSEM + 1))
                for u in range(gg, gg + k):
                    sl = u % RING
                    if u >= RING:
                        # ACT consumed the product unit previously in this slot
                        vec.wait_ge(s_a, u - RING + 1)
                    vec.tensor_mul(
                        pb[:, sl * L : sl * L + F],
                        xb[:, sl * F : (sl + 1) * F],
                        w_t,
                    ).then_inc(s_v, 1)
                    if u == T_UNITS - 1:
                        # final unit: reduce on DVE too, skipping the ACT hop
                        # (wait for our own mult's publication first)
                        vec.wait_ge(s_v, T_UNITS)
                        vec.tensor_reduce(
                            res[:, UNITS - 1 : UNITS],
                            pb[:, sl * L : (sl + 1) * L],
                            axis=mybir.AxisListType.X,
                            op=mybir.AluOpType.add,
                        ).then_inc(s_a, 1)

        @block.scalar
        def _(act: bass.BassEngine):
            from concourse import mybir as mb

            act.wait_ge(s_b, 16)
            for u in range(T_UNITS - 1):
                sl = u % RING
                rc = u % UNITS
                act.wait_ge(s_v, u + 1)
                if u >= RING:
                    # our own in-place writes to this slot were published via
                    # s_a; lagging wait (never stalls in steady state)
                    act.wait_ge(s_a, u - RING + 1)
                if u % RPP == 0 and u // RPP >= 4:
                    # res block reused (reps>1): its output DMA must be done
                    bl = u // RPP
                    act.wait_ge(s_o[bl % NOSEM], 16 * ((bl - 4) // NOSEM + 1))
                act.activation(
                    pb[:, sl * L : (sl + 1) * L],
                    pb[:, sl * L : (sl + 1) * L],
                    mb.ActivationFunctionType.Copy,
                    accum_out=res[:, rc : rc + 1],
                ).then_inc(s_a, 1)

        @block.gpsimd
        def _(gp: bass.BassEngine):
            for bl in range(n_blocks - 1):
                gp.wait_ge(s_a, RPP * (bl + 1))
                if bl >= NOSEM:
                    gp.wait_ge(s_o[bl % NOSEM], 16 * (bl // NOSEM))
                h = (bl % 4) * RPP
                gp.dma_start(
                    out=ys[(bl % 4) * P : (bl % 4 + 1) * P, :],
                    in_=res[:, h : h + RPP],
                ).then_inc(s_o[bl % NOSEM], 16)

    _cache[key] = nc
    return nc


def _make_in_maps(x, lhs, rhs, W, b):
    weff = _fold_weights(lhs, rhs, W)
    wf = np.ascontiguousarray(np.broadcast_to(weff, (P, F)))
    bval = np.float32(np.asarray(b, np.float32).reshape(-1)[0])
    bs = np.full((P, RING), bval, np.float32)
    xr = np.ascontiguousarray(np.asarray(x, np.float32).reshape(B, F))
    in_maps = []
    for c in range(N_CORES):
        shard = xr[c * B_PER : (c + 1) * B_PER].reshape(NPL, LINE)
        in_maps.append({"xs": shard, "wf": wf, "bs": bs})
    return in_maps


def _run(x, lhs, rhs, W, b, reps=1, **kwargs):
    from concourse.bass_utils import run_bass_kernel_spmd

    nc = _build_program(reps)
    in_maps = _make_in_maps(x, lhs, rhs, W, b)
    br = run_bass_kernel_spmd(nc, in_maps, list(range(N_CORES)), **kwargs)
    y = np.concatenate([r["ys"].reshape(B_PER) for r in br.results])
    return y.reshape(B, 1).astype(np.float32), br


def kernel(x, lhs, rhs, W, b):
    y, _ = _run(x, lhs, rhs, W, b)
    return y
